# revision 26
# baseline (speedup 1.0000x reference)
"""Trainium2 Bass kernel for nn_MultiHeadAttention_36051955483000.

Full-shape contract: kernel(**inputs) takes the complete fp32 tensors
(q,k,v: [4,2048,1024]; Wq/Wk/Wv/Wo: [1024,1024]; biases [1024]) and
returns the full [4,2048,1024] fp32 output.

Sharding (8 NeuronCores): core = 2*b + g for batch b in 0..3 and
head-group g in {0,1}. Each core computes 8 of the 16 heads for one
batch, then a pairwise AllGather and the output projection for its 512
output features.

v4 design (single fused instruction stream, scheduler-aware):
- PE warmup matmuls at t=0 release the HAM clock gate early.
- Input DMA priority xq -> xk -> xv: scores/exp start the moment kt
  lands; 6 PT buffers let the exp pipeline run ~6 score-groups ahead
  while the V path (xv + V-projection) is still landing.
- The scalar engine (softmax exp over ~19M elements) paces attention;
  projection and output-projection matmuls are woven as FILLER between
  attention score-groups (the ready-first Tile scheduler interleaves
  them into tensor-engine idle slots).
- V is augmented with a 65th ones-column per (kblock, head); the PV
  matmul emits softmax denominators in PSUM row 64. Epilogue: 1-row
  copy on the otherwise-idle GpSimd engine, broadcast matmul to a
  base-0 PSUM tile, custom-DVE reciprocal (base-0 only!), two
  multiplies.
- Band-group dead halves are never streamed through PV, never memset.
- AllGather per (head-pair, token-half) overlapped with attention; the
  final (e3,t1) collective is split in two so the tail only waits for
  a 128KB transfer. Output projection t=0 half runs as filler inside
  the last head-pair block. tile_wait_until pins gather-dependent
  loads late so the scheduler cannot hoist their waits ahead of
  collective triggers.
"""

import numpy as np
import ml_dtypes

B, N, D, H = 4, 2048, 1024, 16
DH = D // H            # 64
HG = H // 2            # 8 heads per core
FG = D // 2            # 512 features per head-group
N_CORES = 8
QC = 256               # query-chunk width
NKB = N // 128         # 16 key blocks

BF16 = ml_dtypes.bfloat16
# chunked-AllGather feature-block order (see _build_program)
PERM = [0, 4, 1, 5, 2, 6, 3, 7]

WARMUP_MMS = 44

_PROG = None


def _build_program():
    from concourse import bacc, tile, mybir

    f32 = mybir.dt.float32
    bf16 = mybir.dt.bfloat16

    nc = bacc.Bacc("TRN2", target_bir_lowering=False, debug=False,
                   num_devices=N_CORES)

    xqT = nc.dram_tensor("xqT", [D, N], bf16, kind="ExternalInput").ap()
    xkT = nc.dram_tensor("xkT", [D, N], bf16, kind="ExternalInput").ap()
    xvT = nc.dram_tensor("xvT", [D, N], bf16, kind="ExternalInput").ap()
    wqT = nc.dram_tensor("wqT", [D, FG], bf16, kind="ExternalInput").ap()
    wkT = nc.dram_tensor("wkT", [D, FG], bf16, kind="ExternalInput").ap()
    wvT = nc.dram_tensor("wvT", [D, FG], bf16, kind="ExternalInput").ap()
    woT = nc.dram_tensor("woT", [D, FG], bf16, kind="ExternalInput").ap()
    bq2 = nc.dram_tensor("bq2", [128, 4], f32, kind="ExternalInput").ap()
    bk2 = nc.dram_tensor("bk2", [128, 4], f32, kind="ExternalInput").ap()
    tri01 = nc.dram_tensor("tri01", [128, 128], bf16, kind="ExternalInput").ap()
    y = nc.dram_tensor("y", [N, FG], f32, kind="ExternalOutput").ap()

    add = mybir.AluOpType.add
    mult = mybir.AluOpType.mult
    Exp = mybir.ActivationFunctionType.Exp

    with tile.TileContext(nc) as tc:
        with (
            tc.tile_pool(name="consts", bufs=1) as consts,
            tc.tile_pool(name="dram", bufs=1, space="DRAM") as dram,
            tc.tile_pool(name="xin", bufs=24) as xin,
            tc.tile_pool(name="pt", bufs=6) as ptp,
            tc.tile_pool(name="bcs", bufs=2) as bcsp,
            tc.tile_pool(name="ysb", bufs=2) as ysbp,
            tc.tile_pool(name="sg", bufs=2, space="PSUM") as sgp,
            tc.tile_pool(name="ot", bufs=2, space="PSUM") as otp,
            tc.tile_pool(name="pp", bufs=2, space="PSUM") as pp,
        ):
            wq_sb = consts.tile([128, 8 * FG], bf16, tag="wq")
            wk_sb = consts.tile([128, 8 * FG], bf16, tag="wk")
            wv_sb = consts.tile([128, 8 * FG], bf16, tag="wv")
            wo_sb = consts.tile([128, 8 * FG], bf16, tag="wo")
            qt_sb = consts.tile([128, 4 * N], bf16, tag="qt")
            kt_sb = consts.tile([128, 4 * N], bf16, tag="kt")
            # per (kblock, head): cols 0:64 = V^T block, col 64 = ones
            vaug = consts.tile([128, NKB * HG * 65], bf16, tag="vaug")
            xtown = consts.tile([128, 2048], bf16, tag="xtown")
            warm_r = consts.tile([128, 512], bf16, tag="warm")
            ones_sb = consts.tile([128, 64], f32, tag="ones")
            bq_sb = consts.tile([128, 4], f32, tag="bq")
            bk_sb = consts.tile([128, 4], f32, tag="bk")
            tri_sb = consts.tile([128, 128], bf16, tag="tri")

            vaug_v = vaug[:, :].rearrange("p (t h c) -> p t h c",
                                          t=NKB, h=HG, c=65)

            cc_in = [[dram.tile([128, N // 2], bf16, name=f"cc_in{e}_{t}",
                                tag=f"cci{e}_{t}") for t in range(2)]
                     for e in range(4)]
            cc_out = [[dram.tile([256, N // 2], bf16, name=f"cc_out{e}_{t}",
                                 tag=f"cco{e}_{t}") for t in range(2)]
                      for e in range(4)]
            # the final (e3, t1) gather is split into two quarter ops
            cc_in3 = [dram.tile([128, N // 4], bf16, name=f"cc_in3q{i}",
                                tag=f"cci3q{i}") for i in range(2)]
            cc_out3 = [dram.tile([256, N // 4], bf16, name=f"cc_out3q{i}",
                                 tag=f"cco3q{i}") for i in range(2)]

            # ---- DMA triggers ----
            nc.gpsimd.dma_start(bq_sb[:], bq2[:])
            nc.gpsimd.dma_start(bk_sb[:], bk2[:])
            nc.gpsimd.dma_start(tri_sb[:], tri01[:])
            for db in range(8):
                nc.gpsimd.dma_start(wv_sb[:, 512 * db:512 * db + 512],
                                    wvT[128 * db:128 * db + 128, :])
            # scalar queue (idle until exp starts): wq, wk
            for db in range(8):
                nc.scalar.dma_start(wq_sb[:, 512 * db:512 * db + 512],
                                    wqT[128 * db:128 * db + 128, :])
            for db in range(8):
                nc.scalar.dma_start(wk_sb[:, 512 * db:512 * db + 512],
                                    wkT[128 * db:128 * db + 128, :])
            # sync queue: the big x stream, attention-critical first
            xq = [xin.tile([128, N], bf16, tag="xin", name=f"xq{db}")
                  for db in range(8)]
            for db in range(8):
                nc.sync.dma_start(xq[db][:], xqT[128 * db:128 * db + 128, :])
            xk = [xin.tile([128, N], bf16, tag="xin", name=f"xk{db}")
                  for db in range(8)]
            for db in range(8):
                nc.sync.dma_start(xk[db][:], xkT[128 * db:128 * db + 128, :])
            xv = [xin.tile([128, N], bf16, tag="xin", name=f"xv{db}")
                  for db in range(8)]
            for db in range(8):
                nc.sync.dma_start(xv[db][:], xvT[128 * db:128 * db + 128, :])

            nc.vector.memset(warm_r[:, :], 1.0)
            nc.vector.memset(ones_sb[:, :], 1.0)
            nc.vector.memset(vaug_v[:, :, :, 64:65], 1.0)

            # ---- PE warmup: release the HAM clock gate before real work
            wps = pp.tile([128, 512], f32, tag="pp", name="warm")
            for _ in range(WARMUP_MMS):
                nc.tensor.matmul(wps[:], lhsT=warm_r[:, 0:128], rhs=warm_r[:],
                                 start=True, stop=True)

            # ---- unit emitters ----
            def QU(e, tcx):
                ps = pp.tile([128, 512], f32, tag="pp", name=f"q{e}{tcx}")
                for db in range(8):
                    nc.tensor.matmul(
                        ps[:],
                        lhsT=wq_sb[:, 512 * db + 128 * e:512 * db + 128 * e + 128],
                        rhs=xq[db][:, 512 * tcx:512 * tcx + 512],
                        start=(db == 0), stop=(db == 7))
                nc.vector.tensor_scalar(
                    qt_sb[:, 2048 * e + 512 * tcx:2048 * e + 512 * tcx + 512],
                    ps[:], bq_sb[:, e:e + 1], None, add)

            def KU(e, tcx):
                ps = pp.tile([128, 512], f32, tag="pp", name=f"k{e}{tcx}")
                for db in range(8):
                    nc.tensor.matmul(
                        ps[:],
                        lhsT=wk_sb[:, 512 * db + 128 * e:512 * db + 128 * e + 128],
                        rhs=xk[db][:, 512 * tcx:512 * tcx + 512],
                        start=(db == 0), stop=(db == 7))
                nc.vector.tensor_scalar(
                    kt_sb[:, 2048 * e + 512 * tcx:2048 * e + 512 * tcx + 512],
                    ps[:], bk_sb[:, e:e + 1], None, add)

            def VU(tb):
                # V rows for token block tb, all 8 heads
                ps = pp.tile([128, 512], f32, tag="pp", name=f"v{tb}")
                for db in range(8):
                    nc.tensor.matmul(
                        ps[:],
                        lhsT=xv[db][:, 128 * tb:128 * tb + 128],
                        rhs=wv_sb[:, 512 * db:512 * db + 512],
                        start=(db == 0), stop=(db == 7))
                nc.vector.tensor_copy(
                    vaug_v[:, tb, :, 0:64],
                    ps[:, :].rearrange("p (h c) -> p h c", h=8, c=64))

            def OU(t, i, xts):
                # output projection for token block 8*t + i
                ps = pp.tile([128, 512], f32, tag="pp", name=f"o{t}{i}")
                for idx in range(8):
                    nc.tensor.matmul(
                        ps[:],
                        lhsT=xts[idx][:, 128 * i:128 * i + 128],
                        rhs=wo_sb[:, 512 * idx:512 * idx + 512],
                        start=(idx == 0), stop=(idx == 7))
                ysb = ysbp.tile([128, 512], f32, tag="ysb", name=f"ysb{t}{i}")
                nc.vector.tensor_copy(ysb[:], ps[:])
                tb = 8 * t + i
                nc.sync.dma_start(y[128 * tb:128 * tb + 128, :], ysb[:])

            # ---- prologue: tensor work while the x stream lands ----
            for tcx in range(4):
                QU(0, tcx)
            for tcx in range(4):
                KU(0, tcx)
            for tcx in range(4):
                QU(1, tcx)
            for tcx in range(4):
                QU(2, tcx)
            for tb in range(6):
                VU(tb)
            # deferred wo DMA triggers (gpsimd queue is free now)
            for db in range(8):
                nc.gpsimd.dma_start(wo_sb[:, 512 * db:512 * db + 512],
                                    woT[128 * db:128 * db + 128, :])

            # filler thunks per head-pair block
            fillers = {
                0: [lambda tb=tb: VU(tb) for tb in range(6, NKB)]
                   + [lambda t=t: KU(1, t) for t in range(4)],
                1: [lambda t=t: KU(2, t) for t in range(4)],
                2: [lambda t=t: QU(3, t) for t in range(4)]
                   + [lambda t=t: KU(3, t) for t in range(4)],
                3: [],  # filled dynamically with OU(0, *) after the t0 gather
            }

            xt = {0: [], 1: []}

            def load_xt(t, cis):
                for ci in cis:
                    for r2 in range(2):
                        x = xin.tile([128, N], bf16, tag="xin",
                                     name=f"xt{t}_{ci}_{r2}")
                        if t == 1 and ci == 3:
                            nc.gpsimd.dma_start(
                                x[:, 0:512],
                                cc_out3[0][128 * r2:128 * r2 + 128, :])
                            nc.gpsimd.dma_start(
                                x[:, 512:1024],
                                cc_out3[1][128 * r2:128 * r2 + 128, :])
                        else:
                            nc.gpsimd.dma_start(
                                x[:, 0:1024],
                                cc_out[ci][t][128 * r2:128 * r2 + 128, :])
                        xt[t].append(x)

            # ---- attention streams ----
            def emit_group(e, c, gi):
                hb = 2048 * e
                js = [2 * gi, 2 * gi + 1]
                SG = sgp.tile([128, 4 * QC], f32, tag="SG",
                              name=f"SG{e}_{c}_{gi}")
                for m, j in enumerate(js):
                    for half in (0, 1):
                        po = 64 * half
                        off = 512 * half + QC * m
                        kt_j = kt_sb[po:po + 64,
                                     hb + 128 * j:hb + 128 * j + 128]
                        if j <= 2 * c:
                            nc.tensor.matmul(
                                SG[:, off:off + QC], lhsT=kt_j,
                                rhs=qt_sb[po:po + 64,
                                          hb + QC * c:hb + QC * c + QC],
                                start=True, stop=True,
                                skip_group_check=True)
                        else:  # j == 2c+1: front half is dead
                            nc.tensor.matmul(
                                SG[:, off + 128:off + QC],
                                lhsT=kt_j,
                                rhs=qt_sb[po:po + 64,
                                          hb + QC * c + 128:
                                          hb + QC * c + QC],
                                start=True, stop=True,
                                skip_group_check=True)
                PT = ptp.tile([128, 4 * QC], bf16, tag="PT",
                              name=f"PT{e}_{c}_{gi}")
                nc.scalar.activation(PT[:, :], SG[:, :], Exp, scale=0.125)
                if js[-1] == 2 * c + 1:  # band group: mask diagonal blocks
                    for half in (0, 1):
                        off = 512 * half
                        nc.vector.tensor_tensor(
                            PT[:, off:off + 128],
                            PT[:, off:off + 128], tri_sb[:], mult)
                        nc.vector.tensor_tensor(
                            PT[:, off + QC + 128:off + 2 * QC],
                            PT[:, off + QC + 128:off + 2 * QC],
                            tri_sb[:], mult)
                return (c, js, PT)

            ots_by_chunk = {}

            def emit_pv(e, prev):
                pc, pjs, pPT = prev
                if pjs[0] == 0:
                    OT = otp.tile([65, 2 * QC], f32, tag="OT",
                                  name=f"OT{e}_{pc}")
                    ots_by_chunk[pc] = OT
                OT = ots_by_chunk[pc]
                for m, j in enumerate(pjs):
                    for half in (0, 1):
                        h = 2 * e + half
                        va = vaug_v[:, j, h, 0:65]
                        if j <= 2 * pc:
                            nc.tensor.matmul(
                                OT[:, QC * half:QC * half + QC],
                                lhsT=va,
                                rhs=pPT[:, 512 * half + QC * m:
                                        512 * half + QC * m + QC],
                                # start clears has_written bank-wide: only
                                # the tile's very first matmul may carry it
                                start=(j == 0 and half == 0),
                                stop=(j == 2 * pc + 1),
                                skip_group_check=True)
                        else:  # j == 2pc+1: stream only the live back half
                            nc.tensor.matmul(
                                OT[:, QC * half + 128:QC * half + QC],
                                lhsT=va,
                                rhs=pPT[:, 512 * half + QC * m + 128:
                                        512 * half + QC * m + QC],
                                start=False,
                                stop=(j == 2 * pc + 1),
                                skip_group_check=True)

            def trigger_cc(e, t):
                nc.sync.dma_start(cc_in[e][t][:],
                                  xtown[:, 1024 * t:1024 * t + 1024])
                nc.gpsimd.collective_compute(
                    "AllGather",
                    mybir.AluOpType.bypass,
                    replica_groups=[[0, 1], [2, 3], [4, 5], [6, 7]],
                    ins=[cc_in[e][t].opt()],
                    outs=[cc_out[e][t].opt()],
                )

            def trigger_cc3q(i):
                nc.sync.dma_start(cc_in3[i][:],
                                  xtown[:, 1024 + 512 * i:1536 + 512 * i])
                nc.gpsimd.collective_compute(
                    "AllGather",
                    mybir.AluOpType.bypass,
                    replica_groups=[[0, 1], [2, 3], [4, 5], [6, 7]],
                    ins=[cc_in3[i].opt()],
                    outs=[cc_out3[i].opt()],
                )

            def emit_epilogue(e, pc):
                OT = ots_by_chunk.pop(pc)
                t = pc // 4
                xcol = 1024 * t + QC * (pc % 4)
                # denominator row 64 -> broadcast via PE to a base-0 PSUM
                # tile (custom-DVE recip requires base-0 operands)
                dn = bcsp.tile([128, 2 * QC], f32, tag="bcs",
                               name=f"dn{e}_{pc}")
                nc.vector.tensor_copy(dn[64:65, :], OT[64:65, :])
                bc = pp.tile([128, 2 * QC], f32, tag="pp", name=f"bc{e}_{pc}")
                nc.tensor.matmul(bc[0:64, :], lhsT=ones_sb[64:65, :],
                                 rhs=dn[64:65, :], start=True, stop=True)
                bcs = bcsp.tile([64, 2 * QC], f32, tag="bcs",
                                name=f"bcs{e}_{pc}")
                nc.vector.reciprocal_approx_fast(bcs[:, :], bc[0:64, :])
                for half in (0, 1):
                    nc.vector.tensor_tensor(
                        xtown[64 * half:64 * half + 64, xcol:xcol + QC],
                        OT[0:64, QC * half:QC * half + QC],
                        bcs[:, QC * half:QC * half + QC], mult)
                if e == 3 and t == 1:
                    if pc == 5:
                        trigger_cc3q(0)
                    elif pc == 7:
                        trigger_cc3q(1)
                elif pc in (3, 7):
                    trigger_cc(e, t)
                if e == 3 and pc == 3:
                    # final t0 gather fired: its xt pair + the t0 out-proj
                    # queue as filler for chunks 4-7
                    with tc.tile_wait_until(0.215):
                        load_xt(0, [3])
                    fillers[3].extend(
                        [lambda i=i: OU(0, i, xt[0]) for i in range(8)])

            def attention_stream(e):
                groups = [(c, gi) for c in range(8) for gi in range(c + 1)]
                fl = fillers[e]
                fill_idx = 0
                prev = None
                for idx in range(len(groups) + 1):
                    cur = None
                    if idx < len(groups):
                        c, gi = groups[idx]
                        cur = emit_group(e, c, gi)
                    if prev is not None:
                        emit_pv(e, prev)
                        pc, pjs, _ = prev
                        if pjs[-1] == 2 * pc + 1:
                            emit_epilogue(e, pc)
                    # weave fillers evenly across the group stream
                    # (fillers[3] grows mid-stream after the t0 gather)
                    target = (len(fl) * (idx + 1)) // (len(groups) + 1)
                    if e == 3:
                        target = max(0, min(len(fl), (idx + 1 - 14) * 2))
                    while fill_idx < min(target, len(fl)):
                        fl[fill_idx]()
                        fill_idx += 1
                    prev = cur
                while fill_idx < len(fl):
                    fl[fill_idx]()
                    fill_idx += 1

            attention_stream(0)
            attention_stream(1)
            attention_stream(2)
            # gather-dependent loads for the out-projection: all gathers for
            # e0..e2 are done by e3; pin them at e3 logical time so the
            # scheduler cannot hoist their waits ahead of e3's collectives
            with tc.tile_wait_until(0.195):
                load_xt(0, [0, 1, 2])
                load_xt(1, [0, 1, 2])
            attention_stream(3)

            # ---- tail: final quarter gathers -> last xt pair -> t1 out-proj
            with tc.tile_wait_until(0.235):
                load_xt(1, [3])
                for i in range(8):
                    OU(1, i, xt[1])

    nc.compile()
    return nc


def _program():
    global _PROG
    if _PROG is None:
        _PROG = _build_program()
    return _PROG


def _host_inputs(q, k, v, Wq, bq, Wk, bk, Wv, bv, Wo):
    qb = np.asarray(q, np.float32).astype(BF16)
    kb = np.asarray(k, np.float32).astype(BF16)
    vb = np.asarray(v, np.float32).astype(BF16)
    xqT = [np.ascontiguousarray(qb[b].T) for b in range(B)]
    xkT = [np.ascontiguousarray(kb[b].T) for b in range(B)]
    xvT = [np.ascontiguousarray(vb[b].T) for b in range(B)]

    def wslice(W, g):
        return np.ascontiguousarray(
            np.asarray(W, np.float32)[FG * g:FG * (g + 1), :].T).astype(BF16)

    wqg = [wslice(Wq, g) for g in range(2)]
    wkg = [wslice(Wk, g) for g in range(2)]
    wvg = [wslice(Wv, g) for g in range(2)]

    def woslice(g):
        wt = np.ascontiguousarray(
            np.asarray(Wo, np.float32)[FG * g:FG * (g + 1), :].T).astype(BF16)
        # permute 128-row input-feature blocks to the chunked-AG order
        return np.ascontiguousarray(
            wt.reshape(8, 128, FG)[PERM].reshape(D, FG))

    wog = [woslice(g) for g in range(2)]

    def bslice(bvec, g):
        return np.ascontiguousarray(
            np.asarray(bvec, np.float32)[FG * g:FG * (g + 1)]
            .reshape(4, 128).T)

    bqg = [bslice(bq, g) for g in range(2)]
    bkg = [bslice(bk, g) for g in range(2)]

    kk, qq = np.meshgrid(np.arange(128), np.arange(128), indexing="ij")
    tri = np.where(kk <= qq, 1.0, 0.0).astype(BF16)

    in_maps = []
    for core in range(N_CORES):
        b, g = core // 2, core % 2
        in_maps.append({
            "xqT": xqT[b], "xkT": xkT[b], "xvT": xvT[b],
            "wqT": wqg[g], "wkT": wkg[g], "wvT": wvg[g], "woT": wog[g],
            "bq2": bqg[g], "bk2": bkg[g], "tri01": tri,
        })
    return in_maps


def run_sharded(in_maps, trace=False, trace_kwargs=None):
    from concourse.bass_utils import run_bass_kernel_spmd
    nc = _program()
    return run_bass_kernel_spmd(nc, in_maps, core_ids=list(range(N_CORES)),
                                trace=trace, trace_kwargs=trace_kwargs or {})


def kernel(q, k, v, Wq, bq, Wk, bk, Wv, bv, Wo):
    in_maps = _host_inputs(q, k, v, Wq, bq, Wk, bk, Wv, bv, Wo)
    res = run_sharded(in_maps)
    out = np.empty((B, N, D), np.float32)
    for b in range(B):
        out[b, :, 0:FG] = res.results[2 * b]["y"]
        out[b, :, FG:D] = res.results[2 * b + 1]["y"]
    return out


# revision 27
# speedup vs baseline: 1.1294x; 1.1294x over previous
"""Trainium2 Bass kernel for nn_MultiHeadAttention_36051955483000.

Full-shape contract: kernel(**inputs) takes the complete fp32 tensors
(q,k,v: [4,2048,1024]; Wq/Wk/Wv/Wo: [1024,1024]; biases [1024]) and
returns the full [4,2048,1024] fp32 output.

Sharding (8 NeuronCores): core = 2*b + g for batch b in 0..3 and
head-group g in {0,1}. Each core computes 8 of the 16 heads for one
batch, then a pairwise AllGather and the output projection for its 512
output features.

v4 design (single fused instruction stream, scheduler-aware):
- PE warmup matmuls at t=0 release the HAM clock gate early.
- Input DMA priority xq -> xk -> xv: scores/exp start the moment kt
  lands; 6 PT buffers let the exp pipeline run ~6 score-groups ahead
  while the V path (xv + V-projection) is still landing.
- The scalar engine (softmax exp over ~19M elements) paces attention;
  projection and output-projection matmuls are woven as FILLER between
  attention score-groups (the ready-first Tile scheduler interleaves
  them into tensor-engine idle slots).
- V is augmented with a 65th ones-column per (kblock, head); the PV
  matmul emits softmax denominators in PSUM row 64. Epilogue: 1-row
  copy on the otherwise-idle GpSimd engine, broadcast matmul to a
  base-0 PSUM tile, custom-DVE reciprocal (base-0 only!), two
  multiplies.
- Band-group dead halves are never streamed through PV, never memset.
- AllGather per (head-pair, token-half) overlapped with attention; the
  final (e3,t1) collective is split in two so the tail only waits for
  a 128KB transfer. Output projection t=0 half runs as filler inside
  the last head-pair block. tile_wait_until pins gather-dependent
  loads late so the scheduler cannot hoist their waits ahead of
  collective triggers.
"""

import numpy as np
import ml_dtypes

B, N, D, H = 4, 2048, 1024, 16
DH = D // H            # 64
HG = H // 2            # 8 heads per core
FG = D // 2            # 512 features per head-group
N_CORES = 8
QC = 256               # query-chunk width
NKB = N // 128         # 16 key blocks

BF16 = ml_dtypes.bfloat16
# chunked-AllGather feature-block order (see _build_program)
PERM = [0, 4, 1, 5, 2, 6, 3, 7]

WARMUP_MMS = 44

_PROG = None


def _build_program():
    from concourse import bacc, tile, mybir

    f32 = mybir.dt.float32
    bf16 = mybir.dt.bfloat16

    nc = bacc.Bacc("TRN2", target_bir_lowering=False, debug=False,
                   num_devices=N_CORES)

    xqT = nc.dram_tensor("xqT", [D, N], bf16, kind="ExternalInput").ap()
    xkT = nc.dram_tensor("xkT", [D, N], bf16, kind="ExternalInput").ap()
    xvT = nc.dram_tensor("xvT", [D, N], bf16, kind="ExternalInput").ap()
    wqT = nc.dram_tensor("wqT", [D, FG], bf16, kind="ExternalInput").ap()
    wkT = nc.dram_tensor("wkT", [D, FG], bf16, kind="ExternalInput").ap()
    wvT = nc.dram_tensor("wvT", [D, FG], bf16, kind="ExternalInput").ap()
    woT = nc.dram_tensor("woT", [D, FG], bf16, kind="ExternalInput").ap()
    bq2 = nc.dram_tensor("bq2", [128, 4], f32, kind="ExternalInput").ap()
    bk2 = nc.dram_tensor("bk2", [128, 4], f32, kind="ExternalInput").ap()
    tri01 = nc.dram_tensor("tri01", [128, 128], bf16, kind="ExternalInput").ap()
    y = nc.dram_tensor("y", [N, FG], f32, kind="ExternalOutput").ap()

    add = mybir.AluOpType.add
    mult = mybir.AluOpType.mult
    Exp = mybir.ActivationFunctionType.Exp

    with tile.TileContext(nc) as tc:
        with (
            tc.tile_pool(name="consts", bufs=1) as consts,
            tc.tile_pool(name="dram", bufs=1, space="DRAM") as dram,
            tc.tile_pool(name="xin", bufs=24) as xin,
            tc.tile_pool(name="pt", bufs=2) as ptp,
            tc.tile_pool(name="bcs", bufs=2) as bcsp,
            tc.tile_pool(name="ysb", bufs=1) as ysbp,
            tc.tile_pool(name="sg", bufs=2, space="PSUM") as sgp,
            tc.tile_pool(name="ot", bufs=2, space="PSUM") as otp,
            tc.tile_pool(name="pp", bufs=2, space="PSUM") as pp,
        ):
            wq_sb = consts.tile([128, 8 * FG], bf16, tag="wq")
            wk_sb = consts.tile([128, 8 * FG], bf16, tag="wk")
            wv_sb = consts.tile([128, 8 * FG], bf16, tag="wv")
            wo_sb = consts.tile([128, 8 * FG], bf16, tag="wo")
            qt_sb = consts.tile([128, 4 * N], bf16, tag="qt")
            kt_sb = consts.tile([128, 4 * N], bf16, tag="kt")
            # per (kblock, head): cols 0:64 = V^T block, 64:128 = ones
            vaug = consts.tile([128, NKB * HG * 128], bf16, tag="vaug")
            xtown = consts.tile([128, 2048], bf16, tag="xtown")
            warm_r = consts.tile([128, 512], bf16, tag="warm")
            bq_sb = consts.tile([128, 4], f32, tag="bq")
            bk_sb = consts.tile([128, 4], f32, tag="bk")
            tri_sb = consts.tile([128, 128], bf16, tag="tri")

            vaug_v = vaug[:, :].rearrange("p (t h c) -> p t h c",
                                          t=NKB, h=HG, c=128)

            cc_in = [[dram.tile([128, N // 2], bf16, name=f"cc_in{e}_{t}",
                                tag=f"cci{e}_{t}") for t in range(2)]
                     for e in range(4)]
            cc_out = [[dram.tile([256, N // 2], bf16, name=f"cc_out{e}_{t}",
                                 tag=f"cco{e}_{t}") for t in range(2)]
                      for e in range(4)]
            # the final (e3, t1) gather is split into two quarter ops
            cc_in3 = [dram.tile([128, N // 4], bf16, name=f"cc_in3q{i}",
                                tag=f"cci3q{i}") for i in range(2)]
            cc_out3 = [dram.tile([256, N // 4], bf16, name=f"cc_out3q{i}",
                                 tag=f"cco3q{i}") for i in range(2)]

            # ---- DMA triggers ----
            nc.gpsimd.dma_start(bq_sb[:], bq2[:])
            nc.gpsimd.dma_start(bk_sb[:], bk2[:])
            nc.gpsimd.dma_start(tri_sb[:], tri01[:])
            for db in range(8):
                nc.gpsimd.dma_start(wv_sb[:, 512 * db:512 * db + 512],
                                    wvT[128 * db:128 * db + 128, :])
            # scalar queue (idle until exp starts): wq, wk
            for db in range(8):
                nc.scalar.dma_start(wq_sb[:, 512 * db:512 * db + 512],
                                    wqT[128 * db:128 * db + 128, :])
            for db in range(8):
                nc.scalar.dma_start(wk_sb[:, 512 * db:512 * db + 512],
                                    wkT[128 * db:128 * db + 128, :])
            # sync queue: the big x stream, attention-critical first
            xq = [xin.tile([128, N], bf16, tag="xin", name=f"xq{db}")
                  for db in range(8)]
            for db in range(8):
                nc.sync.dma_start(xq[db][:], xqT[128 * db:128 * db + 128, :])
            xk = [xin.tile([128, N], bf16, tag="xin", name=f"xk{db}")
                  for db in range(8)]
            for db in range(8):
                nc.sync.dma_start(xk[db][:], xkT[128 * db:128 * db + 128, :])
            xv = [xin.tile([128, N], bf16, tag="xin", name=f"xv{db}")
                  for db in range(8)]
            for db in range(8):
                nc.sync.dma_start(xv[db][:], xvT[128 * db:128 * db + 128, :])

            nc.vector.memset(warm_r[:, :], 1.0)
            nc.vector.memset(vaug_v[:, :, :, 64:128], 1.0)

            # ---- PE warmup: release the HAM clock gate before real work
            wps = pp.tile([128, 512], f32, tag="pp", name="warm")
            for _ in range(WARMUP_MMS):
                nc.tensor.matmul(wps[:], lhsT=warm_r[:, 0:128], rhs=warm_r[:],
                                 start=True, stop=True)

            # ---- unit emitters ----
            def QU(e, tcx):
                ps = pp.tile([128, 512], f32, tag="pp", name=f"q{e}{tcx}")
                for db in range(8):
                    nc.tensor.matmul(
                        ps[:],
                        lhsT=wq_sb[:, 512 * db + 128 * e:512 * db + 128 * e + 128],
                        rhs=xq[db][:, 512 * tcx:512 * tcx + 512],
                        start=(db == 0), stop=(db == 7))
                nc.vector.tensor_scalar(
                    qt_sb[:, 2048 * e + 512 * tcx:2048 * e + 512 * tcx + 512],
                    ps[:], bq_sb[:, e:e + 1], None, add)

            def KU(e, tcx):
                ps = pp.tile([128, 512], f32, tag="pp", name=f"k{e}{tcx}")
                for db in range(8):
                    nc.tensor.matmul(
                        ps[:],
                        lhsT=wk_sb[:, 512 * db + 128 * e:512 * db + 128 * e + 128],
                        rhs=xk[db][:, 512 * tcx:512 * tcx + 512],
                        start=(db == 0), stop=(db == 7))
                nc.vector.tensor_scalar(
                    kt_sb[:, 2048 * e + 512 * tcx:2048 * e + 512 * tcx + 512],
                    ps[:], bk_sb[:, e:e + 1], None, add)

            def VU(tb):
                # V rows for token block tb, all 8 heads
                ps = pp.tile([128, 512], f32, tag="pp", name=f"v{tb}")
                for db in range(8):
                    nc.tensor.matmul(
                        ps[:],
                        lhsT=xv[db][:, 128 * tb:128 * tb + 128],
                        rhs=wv_sb[:, 512 * db:512 * db + 512],
                        start=(db == 0), stop=(db == 7))
                nc.vector.tensor_copy(
                    vaug_v[:, tb, :, 0:64],
                    ps[:, :].rearrange("p (h c) -> p h c", h=8, c=64))

            def OU(t, i, xts):
                # output projection for token block 8*t + i
                ps = pp.tile([128, 512], f32, tag="pp", name=f"o{t}{i}")
                for idx in range(8):
                    nc.tensor.matmul(
                        ps[:],
                        lhsT=xts[idx][:, 128 * i:128 * i + 128],
                        rhs=wo_sb[:, 512 * idx:512 * idx + 512],
                        start=(idx == 0), stop=(idx == 7))
                ysb = ysbp.tile([128, 512], f32, tag="ysb", name=f"ysb{t}{i}")
                nc.vector.tensor_copy(ysb[:], ps[:])
                tb = 8 * t + i
                nc.sync.dma_start(y[128 * tb:128 * tb + 128, :], ysb[:])

            # ---- prologue: tensor work while the x stream lands ----
            for tcx in range(4):
                QU(0, tcx)
            for tcx in range(4):
                KU(0, tcx)
            for tcx in range(4):
                QU(1, tcx)
            for tcx in range(4):
                QU(2, tcx)
            for tb in range(6):
                VU(tb)
            # deferred wo DMA triggers (gpsimd queue is free now)
            for db in range(8):
                nc.gpsimd.dma_start(wo_sb[:, 512 * db:512 * db + 512],
                                    woT[128 * db:128 * db + 128, :])

            # filler thunks per head-pair block
            fillers = {
                0: [lambda tb=tb: VU(tb) for tb in range(6, NKB)]
                   + [lambda t=t: KU(1, t) for t in range(4)],
                1: [lambda t=t: KU(2, t) for t in range(4)],
                2: [lambda t=t: QU(3, t) for t in range(4)]
                   + [lambda t=t: KU(3, t) for t in range(4)],
                3: [],  # filled dynamically with OU(0, *) after the t0 gather
            }

            xt = {0: [], 1: []}

            def load_xt(t, cis):
                for ci in cis:
                    for r2 in range(2):
                        x = xin.tile([128, N], bf16, tag="xin",
                                     name=f"xt{t}_{ci}_{r2}")
                        if t == 1 and ci == 3:
                            nc.gpsimd.dma_start(
                                x[:, 0:512],
                                cc_out3[0][128 * r2:128 * r2 + 128, :])
                            nc.gpsimd.dma_start(
                                x[:, 512:1024],
                                cc_out3[1][128 * r2:128 * r2 + 128, :])
                        else:
                            nc.gpsimd.dma_start(
                                x[:, 0:1024],
                                cc_out[ci][t][128 * r2:128 * r2 + 128, :])
                        xt[t].append(x)

            # ---- attention streams ----
            def emit_group(e, c, gi):
                hb = 2048 * e
                js = [2 * gi, 2 * gi + 1]
                SG = sgp.tile([128, 4 * QC], f32, tag="SG",
                              name=f"SG{e}_{c}_{gi}")
                for m, j in enumerate(js):
                    for half in (0, 1):
                        po = 64 * half
                        off = 512 * half + QC * m
                        kt_j = kt_sb[po:po + 64,
                                     hb + 128 * j:hb + 128 * j + 128]
                        if j <= 2 * c:
                            nc.tensor.matmul(
                                SG[:, off:off + QC], lhsT=kt_j,
                                rhs=qt_sb[po:po + 64,
                                          hb + QC * c:hb + QC * c + QC],
                                start=True, stop=True,
                                skip_group_check=True)
                        else:  # j == 2c+1: front half is dead
                            nc.tensor.matmul(
                                SG[:, off + 128:off + QC],
                                lhsT=kt_j,
                                rhs=qt_sb[po:po + 64,
                                          hb + QC * c + 128:
                                          hb + QC * c + QC],
                                start=True, stop=True,
                                skip_group_check=True)
                PT = ptp.tile([128, 4 * QC], bf16, tag="PT",
                              name=f"PT{e}_{c}_{gi}")
                nc.scalar.activation(PT[:, :], SG[:, :], Exp, scale=0.125)
                if js[-1] == 2 * c + 1:  # band group: mask diagonal blocks
                    for half in (0, 1):
                        off = 512 * half
                        nc.vector.tensor_tensor(
                            PT[:, off:off + 128],
                            PT[:, off:off + 128], tri_sb[:], mult)
                        nc.vector.tensor_tensor(
                            PT[:, off + QC + 128:off + 2 * QC],
                            PT[:, off + QC + 128:off + 2 * QC],
                            tri_sb[:], mult)
                return (c, js, PT)

            ots_by_chunk = {}

            def emit_pv(e, prev):
                pc, pjs, pPT = prev
                if pjs[0] == 0:
                    OT = otp.tile([128, 2 * QC], f32, tag="OT",
                                  name=f"OT{e}_{pc}")
                    ots_by_chunk[pc] = OT
                OT = ots_by_chunk[pc]
                for m, j in enumerate(pjs):
                    for half in (0, 1):
                        h = 2 * e + half
                        va = vaug_v[:, j, h, 0:128]
                        if j <= 2 * pc:
                            nc.tensor.matmul(
                                OT[:, QC * half:QC * half + QC],
                                lhsT=va,
                                rhs=pPT[:, 512 * half + QC * m:
                                        512 * half + QC * m + QC],
                                # start clears has_written bank-wide: only
                                # the tile's very first matmul may carry it
                                start=(j == 0 and half == 0),
                                stop=(j == 2 * pc + 1),
                                skip_group_check=True)
                        else:  # j == 2pc+1: stream only the live back half
                            nc.tensor.matmul(
                                OT[:, QC * half + 128:QC * half + QC],
                                lhsT=va,
                                rhs=pPT[:, 512 * half + QC * m + 128:
                                        512 * half + QC * m + QC],
                                start=False,
                                stop=(j == 2 * pc + 1),
                                skip_group_check=True)

            def trigger_cc(e, t):
                nc.sync.dma_start(cc_in[e][t][:],
                                  xtown[:, 1024 * t:1024 * t + 1024])
                nc.gpsimd.collective_compute(
                    "AllGather",
                    mybir.AluOpType.bypass,
                    replica_groups=[[0, 1], [2, 3], [4, 5], [6, 7]],
                    ins=[cc_in[e][t].opt()],
                    outs=[cc_out[e][t].opt()],
                )

            def trigger_cc3q(i):
                nc.sync.dma_start(cc_in3[i][:],
                                  xtown[:, 1024 + 512 * i:1536 + 512 * i])
                nc.gpsimd.collective_compute(
                    "AllGather",
                    mybir.AluOpType.bypass,
                    replica_groups=[[0, 1], [2, 3], [4, 5], [6, 7]],
                    ins=[cc_in3[i].opt()],
                    outs=[cc_out3[i].opt()],
                )

            def emit_epilogue(e, pc):
                OT = ots_by_chunk.pop(pc)
                t = pc // 4
                xcol = 1024 * t + QC * (pc % 4)
                # custom-DVE recip mislowers base-partition-64 operands:
                # stage the replicated d rows at base 0 first
                dtm = bcsp.tile([64, 2 * QC], f32, tag="bcs",
                                name=f"dtm{e}_{pc}")
                nc.vector.tensor_copy(dtm[:, :], OT[64:128, :])
                bcs = bcsp.tile([64, 2 * QC], f32, tag="bcs",
                                name=f"bcs{e}_{pc}")
                nc.vector.reciprocal_approx_fast(bcs[:, :], dtm[:, :])
                for half in (0, 1):
                    nc.vector.tensor_tensor(
                        xtown[64 * half:64 * half + 64, xcol:xcol + QC],
                        OT[0:64, QC * half:QC * half + QC],
                        bcs[:, QC * half:QC * half + QC], mult)
                if e == 3 and t == 1:
                    if pc == 5:
                        trigger_cc3q(0)
                    elif pc == 7:
                        trigger_cc3q(1)
                elif pc in (3, 7):
                    trigger_cc(e, t)
                if e == 3 and pc == 3:
                    # final t0 gather fired: its xt pair + the t0 out-proj
                    # queue as filler for chunks 4-7
                    with tc.tile_wait_until(0.215):
                        load_xt(0, [3])
                    fillers[3].extend(
                        [lambda i=i: OU(0, i, xt[0]) for i in range(8)])

            def attention_stream(e):
                groups = [(c, gi) for c in range(8) for gi in range(c + 1)]
                fl = fillers[e]
                fill_idx = 0
                prev = None
                for idx in range(len(groups) + 1):
                    cur = None
                    if idx < len(groups):
                        c, gi = groups[idx]
                        cur = emit_group(e, c, gi)
                    if prev is not None:
                        emit_pv(e, prev)
                        pc, pjs, _ = prev
                        if pjs[-1] == 2 * pc + 1:
                            emit_epilogue(e, pc)
                    # weave fillers evenly across the group stream
                    # (fillers[3] grows mid-stream after the t0 gather)
                    target = (len(fl) * (idx + 1)) // (len(groups) + 1)
                    if e == 3:
                        target = max(0, min(len(fl), (idx + 1 - 14) * 2))
                    while fill_idx < min(target, len(fl)):
                        fl[fill_idx]()
                        fill_idx += 1
                    prev = cur
                while fill_idx < len(fl):
                    fl[fill_idx]()
                    fill_idx += 1

            attention_stream(0)
            attention_stream(1)
            attention_stream(2)
            # gather-dependent loads for the out-projection: all gathers for
            # e0..e2 are done by e3; pin them at e3 logical time so the
            # scheduler cannot hoist their waits ahead of e3's collectives
            with tc.tile_wait_until(0.195):
                load_xt(0, [0, 1, 2])
                load_xt(1, [0, 1, 2])
            attention_stream(3)

            # ---- tail: final quarter gathers -> last xt pair -> t1 out-proj
            with tc.tile_wait_until(0.235):
                load_xt(1, [3])
                for i in range(8):
                    OU(1, i, xt[1])

    nc.compile()
    return nc


def _program():
    global _PROG
    if _PROG is None:
        _PROG = _build_program()
    return _PROG


def _host_inputs(q, k, v, Wq, bq, Wk, bk, Wv, bv, Wo):
    qb = np.asarray(q, np.float32).astype(BF16)
    kb = np.asarray(k, np.float32).astype(BF16)
    vb = np.asarray(v, np.float32).astype(BF16)
    xqT = [np.ascontiguousarray(qb[b].T) for b in range(B)]
    xkT = [np.ascontiguousarray(kb[b].T) for b in range(B)]
    xvT = [np.ascontiguousarray(vb[b].T) for b in range(B)]

    def wslice(W, g):
        return np.ascontiguousarray(
            np.asarray(W, np.float32)[FG * g:FG * (g + 1), :].T).astype(BF16)

    wqg = [wslice(Wq, g) for g in range(2)]
    wkg = [wslice(Wk, g) for g in range(2)]
    wvg = [wslice(Wv, g) for g in range(2)]

    def woslice(g):
        wt = np.ascontiguousarray(
            np.asarray(Wo, np.float32)[FG * g:FG * (g + 1), :].T).astype(BF16)
        # permute 128-row input-feature blocks to the chunked-AG order
        return np.ascontiguousarray(
            wt.reshape(8, 128, FG)[PERM].reshape(D, FG))

    wog = [woslice(g) for g in range(2)]

    def bslice(bvec, g):
        return np.ascontiguousarray(
            np.asarray(bvec, np.float32)[FG * g:FG * (g + 1)]
            .reshape(4, 128).T)

    bqg = [bslice(bq, g) for g in range(2)]
    bkg = [bslice(bk, g) for g in range(2)]

    kk, qq = np.meshgrid(np.arange(128), np.arange(128), indexing="ij")
    tri = np.where(kk <= qq, 1.0, 0.0).astype(BF16)

    in_maps = []
    for core in range(N_CORES):
        b, g = core // 2, core % 2
        in_maps.append({
            "xqT": xqT[b], "xkT": xkT[b], "xvT": xvT[b],
            "wqT": wqg[g], "wkT": wkg[g], "wvT": wvg[g], "woT": wog[g],
            "bq2": bqg[g], "bk2": bkg[g], "tri01": tri,
        })
    return in_maps


def run_sharded(in_maps, trace=False, trace_kwargs=None):
    from concourse.bass_utils import run_bass_kernel_spmd
    nc = _program()
    return run_bass_kernel_spmd(nc, in_maps, core_ids=list(range(N_CORES)),
                                trace=trace, trace_kwargs=trace_kwargs or {})


def kernel(q, k, v, Wq, bq, Wk, bk, Wv, bv, Wo):
    in_maps = _host_inputs(q, k, v, Wq, bq, Wk, bk, Wv, bv, Wo)
    res = run_sharded(in_maps)
    out = np.empty((B, N, D), np.float32)
    for b in range(B):
        out[b, :, 0:FG] = res.results[2 * b]["y"]
        out[b, :, FG:D] = res.results[2 * b + 1]["y"]
    return out


# revision 28
# speedup vs baseline: 1.1303x; 1.0008x over previous
"""Trainium2 Bass kernel for nn_MultiHeadAttention_36051955483000.

Full-shape contract: kernel(**inputs) takes the complete fp32 tensors
(q,k,v: [4,2048,1024]; Wq/Wk/Wv/Wo: [1024,1024]; biases [1024]) and
returns the full [4,2048,1024] fp32 output.

Sharding (8 NeuronCores): core = 2*b + g for batch b in 0..3 and
head-group g in {0,1}. Each core computes 8 of the 16 heads for one
batch, then a pairwise AllGather and the output projection for its 512
output features.

v4 design (single fused instruction stream, scheduler-aware):
- PE warmup matmuls at t=0 release the HAM clock gate early.
- Input DMA priority xq -> xk -> xv: scores/exp start the moment kt
  lands; 6 PT buffers let the exp pipeline run ~6 score-groups ahead
  while the V path (xv + V-projection) is still landing.
- The scalar engine (softmax exp over ~19M elements) paces attention;
  projection and output-projection matmuls are woven as FILLER between
  attention score-groups (the ready-first Tile scheduler interleaves
  them into tensor-engine idle slots).
- V is augmented with a 65th ones-column per (kblock, head); the PV
  matmul emits softmax denominators in PSUM row 64. Epilogue: 1-row
  copy on the otherwise-idle GpSimd engine, broadcast matmul to a
  base-0 PSUM tile, custom-DVE reciprocal (base-0 only!), two
  multiplies.
- Band-group dead halves are never streamed through PV, never memset.
- AllGather per (head-pair, token-half) overlapped with attention; the
  final (e3,t1) collective is split in two so the tail only waits for
  a 128KB transfer. Output projection t=0 half runs as filler inside
  the last head-pair block. tile_wait_until pins gather-dependent
  loads late so the scheduler cannot hoist their waits ahead of
  collective triggers.
"""

import numpy as np
import ml_dtypes

B, N, D, H = 4, 2048, 1024, 16
DH = D // H            # 64
HG = H // 2            # 8 heads per core
FG = D // 2            # 512 features per head-group
N_CORES = 8
QC = 256               # query-chunk width
NKB = N // 128         # 16 key blocks

BF16 = ml_dtypes.bfloat16
# chunked-AllGather feature-block order (see _build_program)
PERM = [0, 4, 1, 5, 2, 6, 3, 7]

WARMUP_MMS = 44

_PROG = None


def _build_program():
    from concourse import bacc, tile, mybir

    f32 = mybir.dt.float32
    bf16 = mybir.dt.bfloat16

    nc = bacc.Bacc("TRN2", target_bir_lowering=False, debug=False,
                   num_devices=N_CORES)

    xqT = nc.dram_tensor("xqT", [D, N], bf16, kind="ExternalInput").ap()
    xkT = nc.dram_tensor("xkT", [D, N], bf16, kind="ExternalInput").ap()
    xvT = nc.dram_tensor("xvT", [D, N], bf16, kind="ExternalInput").ap()
    wqT = nc.dram_tensor("wqT", [D, FG], bf16, kind="ExternalInput").ap()
    wkT = nc.dram_tensor("wkT", [D, FG], bf16, kind="ExternalInput").ap()
    wvT = nc.dram_tensor("wvT", [D, FG], bf16, kind="ExternalInput").ap()
    woT = nc.dram_tensor("woT", [D, FG], bf16, kind="ExternalInput").ap()
    bq2 = nc.dram_tensor("bq2", [128, 4], f32, kind="ExternalInput").ap()
    bk2 = nc.dram_tensor("bk2", [128, 4], f32, kind="ExternalInput").ap()
    tri01 = nc.dram_tensor("tri01", [128, 128], bf16, kind="ExternalInput").ap()
    y = nc.dram_tensor("y", [N, FG], f32, kind="ExternalOutput").ap()

    add = mybir.AluOpType.add
    mult = mybir.AluOpType.mult
    Exp = mybir.ActivationFunctionType.Exp

    with tile.TileContext(nc) as tc:
        with (
            tc.tile_pool(name="consts", bufs=1) as consts,
            tc.tile_pool(name="dram", bufs=1, space="DRAM") as dram,
            tc.tile_pool(name="xin", bufs=24) as xin,
            tc.tile_pool(name="pt", bufs=2) as ptp,
            tc.tile_pool(name="bcs", bufs=2) as bcsp,
            tc.tile_pool(name="ysb", bufs=1) as ysbp,
            tc.tile_pool(name="sg", bufs=2, space="PSUM") as sgp,
            tc.tile_pool(name="ot", bufs=2, space="PSUM") as otp,
            tc.tile_pool(name="pp", bufs=2, space="PSUM") as pp,
        ):
            wq_sb = consts.tile([128, 8 * FG], bf16, tag="wq")
            wk_sb = consts.tile([128, 8 * FG], bf16, tag="wk")
            wv_sb = consts.tile([128, 8 * FG], bf16, tag="wv")
            wo_sb = consts.tile([128, 8 * FG], bf16, tag="wo")
            qt_sb = consts.tile([128, 4 * N], bf16, tag="qt")
            kt_sb = consts.tile([128, 4 * N], bf16, tag="kt")
            # per (kblock, head): cols 0:64 = V^T block, 64:128 = ones
            vaug = consts.tile([128, NKB * HG * 128], bf16, tag="vaug")
            xtown = consts.tile([128, 2048], bf16, tag="xtown")
            warm_r = consts.tile([128, 512], bf16, tag="warm")
            bq_sb = consts.tile([128, 4], f32, tag="bq")
            bk_sb = consts.tile([128, 4], f32, tag="bk")
            tri_sb = consts.tile([128, 128], bf16, tag="tri")

            vaug_v = vaug[:, :].rearrange("p (t h c) -> p t h c",
                                          t=NKB, h=HG, c=128)

            cc_in = [[dram.tile([128, N // 2], bf16, name=f"cc_in{e}_{t}",
                                tag=f"cci{e}_{t}") for t in range(2)]
                     for e in range(4)]
            cc_out = [[dram.tile([256, N // 2], bf16, name=f"cc_out{e}_{t}",
                                 tag=f"cco{e}_{t}") for t in range(2)]
                      for e in range(4)]
            # the final (e3, t1) gather is split into two quarter ops
            cc_in3 = [dram.tile([128, N // 4], bf16, name=f"cc_in3q{i}",
                                tag=f"cci3q{i}") for i in range(2)]
            cc_out3 = [dram.tile([256, N // 4], bf16, name=f"cc_out3q{i}",
                                 tag=f"cco3q{i}") for i in range(2)]

            # ---- DMA triggers ----
            nc.gpsimd.dma_start(bq_sb[:], bq2[:])
            nc.gpsimd.dma_start(bk_sb[:], bk2[:])
            nc.gpsimd.dma_start(tri_sb[:], tri01[:])
            for db in range(8):
                nc.gpsimd.dma_start(wv_sb[:, 512 * db:512 * db + 512],
                                    wvT[128 * db:128 * db + 128, :])
            # scalar queue (idle until exp starts): wq, wk
            for db in range(8):
                nc.scalar.dma_start(wq_sb[:, 512 * db:512 * db + 512],
                                    wqT[128 * db:128 * db + 128, :])
            for db in range(8):
                nc.scalar.dma_start(wk_sb[:, 512 * db:512 * db + 512],
                                    wkT[128 * db:128 * db + 128, :])
            # sync queue: the big x stream in need-order xq, xv, xk
            xq = [xin.tile([128, N], bf16, tag="xin", name=f"xq{db}")
                  for db in range(8)]
            for db in range(8):
                nc.sync.dma_start(xq[db][:], xqT[128 * db:128 * db + 128, :])
            xv = [xin.tile([128, N], bf16, tag="xin", name=f"xv{db}")
                  for db in range(8)]
            for db in range(8):
                nc.sync.dma_start(xv[db][:], xvT[128 * db:128 * db + 128, :])
            xk = [xin.tile([128, N], bf16, tag="xin", name=f"xk{db}")
                  for db in range(8)]
            for db in range(8):
                nc.sync.dma_start(xk[db][:], xkT[128 * db:128 * db + 128, :])

            nc.vector.memset(warm_r[:, :], 1.0)
            nc.vector.memset(vaug_v[:, :, :, 64:128], 1.0)

            # ---- PE warmup: release the HAM clock gate before real work
            wps = pp.tile([128, 512], f32, tag="pp", name="warm")
            for _ in range(WARMUP_MMS):
                nc.tensor.matmul(wps[:], lhsT=warm_r[:, 0:128], rhs=warm_r[:],
                                 start=True, stop=True)

            # ---- unit emitters ----
            def QU(e, tcx):
                ps = pp.tile([128, 512], f32, tag="pp", name=f"q{e}{tcx}")
                for db in range(8):
                    nc.tensor.matmul(
                        ps[:],
                        lhsT=wq_sb[:, 512 * db + 128 * e:512 * db + 128 * e + 128],
                        rhs=xq[db][:, 512 * tcx:512 * tcx + 512],
                        start=(db == 0), stop=(db == 7))
                nc.vector.tensor_scalar(
                    qt_sb[:, 2048 * e + 512 * tcx:2048 * e + 512 * tcx + 512],
                    ps[:], bq_sb[:, e:e + 1], None, add)

            def KU(e, tcx):
                ps = pp.tile([128, 512], f32, tag="pp", name=f"k{e}{tcx}")
                for db in range(8):
                    nc.tensor.matmul(
                        ps[:],
                        lhsT=wk_sb[:, 512 * db + 128 * e:512 * db + 128 * e + 128],
                        rhs=xk[db][:, 512 * tcx:512 * tcx + 512],
                        start=(db == 0), stop=(db == 7))
                nc.vector.tensor_scalar(
                    kt_sb[:, 2048 * e + 512 * tcx:2048 * e + 512 * tcx + 512],
                    ps[:], bk_sb[:, e:e + 1], None, add)

            def VU(tb):
                # V rows for token block tb, all 8 heads
                ps = pp.tile([128, 512], f32, tag="pp", name=f"v{tb}")
                for db in range(8):
                    nc.tensor.matmul(
                        ps[:],
                        lhsT=xv[db][:, 128 * tb:128 * tb + 128],
                        rhs=wv_sb[:, 512 * db:512 * db + 512],
                        start=(db == 0), stop=(db == 7))
                nc.vector.tensor_copy(
                    vaug_v[:, tb, :, 0:64],
                    ps[:, :].rearrange("p (h c) -> p h c", h=8, c=64))

            def OU(t, i, xts):
                # output projection for token block 8*t + i
                ps = pp.tile([128, 512], f32, tag="pp", name=f"o{t}{i}")
                for idx in range(8):
                    nc.tensor.matmul(
                        ps[:],
                        lhsT=xts[idx][:, 128 * i:128 * i + 128],
                        rhs=wo_sb[:, 512 * idx:512 * idx + 512],
                        start=(idx == 0), stop=(idx == 7))
                ysb = ysbp.tile([128, 512], f32, tag="ysb", name=f"ysb{t}{i}")
                nc.vector.tensor_copy(ysb[:], ps[:])
                tb = 8 * t + i
                nc.sync.dma_start(y[128 * tb:128 * tb + 128, :], ysb[:])

            # ---- prologue: tensor work while the x stream lands ----
            for tcx in range(4):
                QU(0, tcx)
            for tcx in range(4):
                KU(0, tcx)
            for tcx in range(4):
                QU(1, tcx)
            for tcx in range(4):
                QU(2, tcx)
            for tb in range(6):
                VU(tb)
            # deferred wo DMA triggers (gpsimd queue is free now)
            for db in range(8):
                nc.gpsimd.dma_start(wo_sb[:, 512 * db:512 * db + 512],
                                    woT[128 * db:128 * db + 128, :])

            # filler thunks per head-pair block
            fillers = {
                0: [lambda tb=tb: VU(tb) for tb in range(6, NKB)]
                   + [lambda t=t: KU(1, t) for t in range(4)],
                1: [lambda t=t: KU(2, t) for t in range(4)],
                2: [lambda t=t: QU(3, t) for t in range(4)]
                   + [lambda t=t: KU(3, t) for t in range(4)],
                3: [],  # filled dynamically with OU(0, *) after the t0 gather
            }

            xt = {0: [], 1: []}

            def load_xt(t, cis):
                for ci in cis:
                    for r2 in range(2):
                        x = xin.tile([128, N], bf16, tag="xin",
                                     name=f"xt{t}_{ci}_{r2}")
                        if t == 1 and ci == 3:
                            nc.gpsimd.dma_start(
                                x[:, 0:512],
                                cc_out3[0][128 * r2:128 * r2 + 128, :])
                            nc.gpsimd.dma_start(
                                x[:, 512:1024],
                                cc_out3[1][128 * r2:128 * r2 + 128, :])
                        else:
                            nc.gpsimd.dma_start(
                                x[:, 0:1024],
                                cc_out[ci][t][128 * r2:128 * r2 + 128, :])
                        xt[t].append(x)

            # ---- attention streams ----
            def emit_group(e, c, gi):
                hb = 2048 * e
                js = [2 * gi, 2 * gi + 1]
                SG = sgp.tile([128, 4 * QC], f32, tag="SG",
                              name=f"SG{e}_{c}_{gi}")
                for m, j in enumerate(js):
                    for half in (0, 1):
                        po = 64 * half
                        off = 512 * half + QC * m
                        kt_j = kt_sb[po:po + 64,
                                     hb + 128 * j:hb + 128 * j + 128]
                        if j <= 2 * c:
                            nc.tensor.matmul(
                                SG[:, off:off + QC], lhsT=kt_j,
                                rhs=qt_sb[po:po + 64,
                                          hb + QC * c:hb + QC * c + QC],
                                start=True, stop=True,
                                skip_group_check=True)
                        else:  # j == 2c+1: front half is dead
                            nc.tensor.matmul(
                                SG[:, off + 128:off + QC],
                                lhsT=kt_j,
                                rhs=qt_sb[po:po + 64,
                                          hb + QC * c + 128:
                                          hb + QC * c + QC],
                                start=True, stop=True,
                                skip_group_check=True)
                PT = ptp.tile([128, 4 * QC], bf16, tag="PT",
                              name=f"PT{e}_{c}_{gi}")
                nc.scalar.activation(PT[:, :], SG[:, :], Exp, scale=0.125)
                if js[-1] == 2 * c + 1:  # band group: mask diagonal blocks
                    for half in (0, 1):
                        off = 512 * half
                        nc.vector.tensor_tensor(
                            PT[:, off:off + 128],
                            PT[:, off:off + 128], tri_sb[:], mult)
                        nc.vector.tensor_tensor(
                            PT[:, off + QC + 128:off + 2 * QC],
                            PT[:, off + QC + 128:off + 2 * QC],
                            tri_sb[:], mult)
                return (c, js, PT)

            ots_by_chunk = {}

            def emit_pv(e, prev):
                pc, pjs, pPT = prev
                if pjs[0] == 0:
                    OT = otp.tile([128, 2 * QC], f32, tag="OT",
                                  name=f"OT{e}_{pc}")
                    ots_by_chunk[pc] = OT
                OT = ots_by_chunk[pc]
                for m, j in enumerate(pjs):
                    for half in (0, 1):
                        h = 2 * e + half
                        va = vaug_v[:, j, h, 0:128]
                        if j <= 2 * pc:
                            nc.tensor.matmul(
                                OT[:, QC * half:QC * half + QC],
                                lhsT=va,
                                rhs=pPT[:, 512 * half + QC * m:
                                        512 * half + QC * m + QC],
                                # start clears has_written bank-wide: only
                                # the tile's very first matmul may carry it
                                start=(j == 0 and half == 0),
                                stop=(j == 2 * pc + 1),
                                skip_group_check=True)
                        else:  # j == 2pc+1: stream only the live back half
                            nc.tensor.matmul(
                                OT[:, QC * half + 128:QC * half + QC],
                                lhsT=va,
                                rhs=pPT[:, 512 * half + QC * m + 128:
                                        512 * half + QC * m + QC],
                                start=False,
                                stop=(j == 2 * pc + 1),
                                skip_group_check=True)

            def trigger_cc(e, t):
                nc.sync.dma_start(cc_in[e][t][:],
                                  xtown[:, 1024 * t:1024 * t + 1024])
                nc.gpsimd.collective_compute(
                    "AllGather",
                    mybir.AluOpType.bypass,
                    replica_groups=[[0, 1], [2, 3], [4, 5], [6, 7]],
                    ins=[cc_in[e][t].opt()],
                    outs=[cc_out[e][t].opt()],
                )

            def trigger_cc3q(i):
                nc.sync.dma_start(cc_in3[i][:],
                                  xtown[:, 1024 + 512 * i:1536 + 512 * i])
                nc.gpsimd.collective_compute(
                    "AllGather",
                    mybir.AluOpType.bypass,
                    replica_groups=[[0, 1], [2, 3], [4, 5], [6, 7]],
                    ins=[cc_in3[i].opt()],
                    outs=[cc_out3[i].opt()],
                )

            def emit_epilogue(e, pc):
                OT = ots_by_chunk.pop(pc)
                t = pc // 4
                xcol = 1024 * t + QC * (pc % 4)
                # custom-DVE recip mislowers base-partition-64 operands:
                # stage the replicated d rows at base 0 first
                dtm = bcsp.tile([64, 2 * QC], f32, tag="bcs",
                                name=f"dtm{e}_{pc}")
                nc.vector.tensor_copy(dtm[:, :], OT[64:128, :])
                bcs = bcsp.tile([64, 2 * QC], f32, tag="bcs",
                                name=f"bcs{e}_{pc}")
                nc.vector.reciprocal_approx_fast(bcs[:, :], dtm[:, :])
                for half in (0, 1):
                    nc.vector.tensor_tensor(
                        xtown[64 * half:64 * half + 64, xcol:xcol + QC],
                        OT[0:64, QC * half:QC * half + QC],
                        bcs[:, QC * half:QC * half + QC], mult)
                if e == 3 and t == 1:
                    if pc == 5:
                        trigger_cc3q(0)
                    elif pc == 7:
                        trigger_cc3q(1)
                elif pc in (3, 7):
                    trigger_cc(e, t)
                if e == 3 and pc == 3:
                    # final t0 gather fired: its xt pair + the t0 out-proj
                    # queue as filler for chunks 4-7
                    with tc.tile_wait_until(0.215):
                        load_xt(0, [3])
                    fillers[3].extend(
                        [lambda i=i: OU(0, i, xt[0]) for i in range(8)])

            def attention_stream(e):
                groups = [(c, gi) for c in range(8) for gi in range(c + 1)]
                fl = fillers[e]
                fill_idx = 0
                prev = None
                for idx in range(len(groups) + 1):
                    cur = None
                    if idx < len(groups):
                        c, gi = groups[idx]
                        cur = emit_group(e, c, gi)
                    if prev is not None:
                        emit_pv(e, prev)
                        pc, pjs, _ = prev
                        if pjs[-1] == 2 * pc + 1:
                            emit_epilogue(e, pc)
                    # weave fillers evenly across the group stream
                    # (fillers[3] grows mid-stream after the t0 gather)
                    target = (len(fl) * (idx + 1)) // (len(groups) + 1)
                    if e == 3:
                        target = max(0, min(len(fl), (idx + 1 - 14) * 2))
                    while fill_idx < min(target, len(fl)):
                        fl[fill_idx]()
                        fill_idx += 1
                    prev = cur
                while fill_idx < len(fl):
                    fl[fill_idx]()
                    fill_idx += 1

            attention_stream(0)
            attention_stream(1)
            attention_stream(2)
            # gather-dependent loads for the out-projection: all gathers for
            # e0..e2 are done by e3; pin them at e3 logical time so the
            # scheduler cannot hoist their waits ahead of e3's collectives
            with tc.tile_wait_until(0.195):
                load_xt(0, [0, 1, 2])
                load_xt(1, [0, 1, 2])
            attention_stream(3)

            # ---- tail: final quarter gathers -> last xt pair -> t1 out-proj
            with tc.tile_wait_until(0.235):
                load_xt(1, [3])
                for i in range(8):
                    OU(1, i, xt[1])

    nc.compile()
    return nc


def _program():
    global _PROG
    if _PROG is None:
        _PROG = _build_program()
    return _PROG


def _host_inputs(q, k, v, Wq, bq, Wk, bk, Wv, bv, Wo):
    qb = np.asarray(q, np.float32).astype(BF16)
    kb = np.asarray(k, np.float32).astype(BF16)
    vb = np.asarray(v, np.float32).astype(BF16)
    xqT = [np.ascontiguousarray(qb[b].T) for b in range(B)]
    xkT = [np.ascontiguousarray(kb[b].T) for b in range(B)]
    xvT = [np.ascontiguousarray(vb[b].T) for b in range(B)]

    def wslice(W, g):
        return np.ascontiguousarray(
            np.asarray(W, np.float32)[FG * g:FG * (g + 1), :].T).astype(BF16)

    wqg = [wslice(Wq, g) for g in range(2)]
    wkg = [wslice(Wk, g) for g in range(2)]
    wvg = [wslice(Wv, g) for g in range(2)]

    def woslice(g):
        wt = np.ascontiguousarray(
            np.asarray(Wo, np.float32)[FG * g:FG * (g + 1), :].T).astype(BF16)
        # permute 128-row input-feature blocks to the chunked-AG order
        return np.ascontiguousarray(
            wt.reshape(8, 128, FG)[PERM].reshape(D, FG))

    wog = [woslice(g) for g in range(2)]

    def bslice(bvec, g):
        return np.ascontiguousarray(
            np.asarray(bvec, np.float32)[FG * g:FG * (g + 1)]
            .reshape(4, 128).T)

    bqg = [bslice(bq, g) for g in range(2)]
    bkg = [bslice(bk, g) for g in range(2)]

    kk, qq = np.meshgrid(np.arange(128), np.arange(128), indexing="ij")
    tri = np.where(kk <= qq, 1.0, 0.0).astype(BF16)

    in_maps = []
    for core in range(N_CORES):
        b, g = core // 2, core % 2
        in_maps.append({
            "xqT": xqT[b], "xkT": xkT[b], "xvT": xvT[b],
            "wqT": wqg[g], "wkT": wkg[g], "wvT": wvg[g], "woT": wog[g],
            "bq2": bqg[g], "bk2": bkg[g], "tri01": tri,
        })
    return in_maps


def run_sharded(in_maps, trace=False, trace_kwargs=None):
    from concourse.bass_utils import run_bass_kernel_spmd
    nc = _program()
    return run_bass_kernel_spmd(nc, in_maps, core_ids=list(range(N_CORES)),
                                trace=trace, trace_kwargs=trace_kwargs or {})


def kernel(q, k, v, Wq, bq, Wk, bk, Wv, bv, Wo):
    in_maps = _host_inputs(q, k, v, Wq, bq, Wk, bk, Wv, bv, Wo)
    res = run_sharded(in_maps)
    out = np.empty((B, N, D), np.float32)
    for b in range(B):
        out[b, :, 0:FG] = res.results[2 * b]["y"]
        out[b, :, FG:D] = res.results[2 * b + 1]["y"]
    return out


# revision 29
# speedup vs baseline: 1.1438x; 1.0120x over previous
"""Trainium2 Bass kernel for nn_MultiHeadAttention_36051955483000.

Full-shape contract: kernel(**inputs) takes the complete fp32 tensors
(q,k,v: [4,2048,1024]; Wq/Wk/Wv/Wo: [1024,1024]; biases [1024]) and
returns the full [4,2048,1024] fp32 output.

Sharding (8 NeuronCores): core = 2*b + g for batch b in 0..3 and
head-group g in {0,1}. Each core computes 8 of the 16 heads for one
batch, then a pairwise AllGather and the output projection for its 512
output features.

v4 design (single fused instruction stream, scheduler-aware):
- PE warmup matmuls at t=0 release the HAM clock gate early.
- Input DMA priority xq -> xk -> xv: scores/exp start the moment kt
  lands; 6 PT buffers let the exp pipeline run ~6 score-groups ahead
  while the V path (xv + V-projection) is still landing.
- The scalar engine (softmax exp over ~19M elements) paces attention;
  projection and output-projection matmuls are woven as FILLER between
  attention score-groups (the ready-first Tile scheduler interleaves
  them into tensor-engine idle slots).
- V is augmented with a 65th ones-column per (kblock, head); the PV
  matmul emits softmax denominators in PSUM row 64. Epilogue: 1-row
  copy on the otherwise-idle GpSimd engine, broadcast matmul to a
  base-0 PSUM tile, custom-DVE reciprocal (base-0 only!), two
  multiplies.
- Band-group dead halves are never streamed through PV, never memset.
- AllGather per (head-pair, token-half) overlapped with attention; the
  final (e3,t1) collective is split in two so the tail only waits for
  a 128KB transfer. Output projection t=0 half runs as filler inside
  the last head-pair block. tile_wait_until pins gather-dependent
  loads late so the scheduler cannot hoist their waits ahead of
  collective triggers.
"""

import numpy as np
import ml_dtypes

B, N, D, H = 4, 2048, 1024, 16
DH = D // H            # 64
HG = H // 2            # 8 heads per core
FG = D // 2            # 512 features per head-group
N_CORES = 8
QC = 256               # query-chunk width
NKB = N // 128         # 16 key blocks

BF16 = ml_dtypes.bfloat16
# chunked-AllGather feature-block order (see _build_program)
PERM = [0, 4, 1, 5, 2, 6, 3, 7]

WARMUP_MMS = 44

_PROG = None


def _build_program():
    from concourse import bacc, tile, mybir

    f32 = mybir.dt.float32
    bf16 = mybir.dt.bfloat16

    nc = bacc.Bacc("TRN2", target_bir_lowering=False, debug=False,
                   num_devices=N_CORES)

    xqT = nc.dram_tensor("xqT", [D, N], bf16, kind="ExternalInput").ap()
    xkT = nc.dram_tensor("xkT", [D, N], bf16, kind="ExternalInput").ap()
    xvT = nc.dram_tensor("xvT", [D, N], bf16, kind="ExternalInput").ap()
    wqT = nc.dram_tensor("wqT", [D, FG], bf16, kind="ExternalInput").ap()
    wkT = nc.dram_tensor("wkT", [D, FG], bf16, kind="ExternalInput").ap()
    wvT = nc.dram_tensor("wvT", [D, FG], bf16, kind="ExternalInput").ap()
    woT = nc.dram_tensor("woT", [D, FG], bf16, kind="ExternalInput").ap()
    bq2 = nc.dram_tensor("bq2", [128, 4], f32, kind="ExternalInput").ap()
    bk2 = nc.dram_tensor("bk2", [128, 4], f32, kind="ExternalInput").ap()
    tri01 = nc.dram_tensor("tri01", [128, 128], bf16, kind="ExternalInput").ap()
    y = nc.dram_tensor("y", [N, FG], bf16, kind="ExternalOutput").ap()

    add = mybir.AluOpType.add
    mult = mybir.AluOpType.mult
    Exp = mybir.ActivationFunctionType.Exp

    with tile.TileContext(nc) as tc:
        with (
            tc.tile_pool(name="consts", bufs=1) as consts,
            tc.tile_pool(name="dram", bufs=1, space="DRAM") as dram,
            tc.tile_pool(name="xin", bufs=24) as xin,
            tc.tile_pool(name="pt", bufs=2) as ptp,
            tc.tile_pool(name="bcs", bufs=2) as bcsp,
            tc.tile_pool(name="ysb", bufs=2) as ysbp,
            tc.tile_pool(name="sg", bufs=2, space="PSUM") as sgp,
            tc.tile_pool(name="ot", bufs=2, space="PSUM") as otp,
            tc.tile_pool(name="pp", bufs=2, space="PSUM") as pp,
        ):
            wq_sb = consts.tile([128, 8 * FG], bf16, tag="wq")
            wk_sb = consts.tile([128, 8 * FG], bf16, tag="wk")
            wv_sb = consts.tile([128, 8 * FG], bf16, tag="wv")
            wo_sb = consts.tile([128, 8 * FG], bf16, tag="wo")
            qt_sb = consts.tile([128, 4 * N], bf16, tag="qt")
            kt_sb = consts.tile([128, 4 * N], bf16, tag="kt")
            # per (kblock, head): cols 0:64 = V^T block, 64:128 = ones
            vaug = consts.tile([128, NKB * HG * 128], bf16, tag="vaug")
            xtown = consts.tile([128, 2048], bf16, tag="xtown")
            warm_r = consts.tile([128, 512], bf16, tag="warm")
            bq_sb = consts.tile([128, 4], f32, tag="bq")
            bk_sb = consts.tile([128, 4], f32, tag="bk")
            tri_sb = consts.tile([128, 128], bf16, tag="tri")

            vaug_v = vaug[:, :].rearrange("p (t h c) -> p t h c",
                                          t=NKB, h=HG, c=128)

            cc_in = [[dram.tile([128, N // 2], bf16, name=f"cc_in{e}_{t}",
                                tag=f"cci{e}_{t}") for t in range(2)]
                     for e in range(4)]
            cc_out = [[dram.tile([256, N // 2], bf16, name=f"cc_out{e}_{t}",
                                 tag=f"cco{e}_{t}") for t in range(2)]
                      for e in range(4)]
            # the final (e3, t1) gather is split into two quarter ops
            cc_in3 = [dram.tile([128, N // 4], bf16, name=f"cc_in3q{i}",
                                tag=f"cci3q{i}") for i in range(2)]
            cc_out3 = [dram.tile([256, N // 4], bf16, name=f"cc_out3q{i}",
                                 tag=f"cco3q{i}") for i in range(2)]

            # ---- DMA triggers ----
            nc.gpsimd.dma_start(bq_sb[:], bq2[:])
            nc.gpsimd.dma_start(bk_sb[:], bk2[:])
            nc.gpsimd.dma_start(tri_sb[:], tri01[:])
            for db in range(8):
                nc.gpsimd.dma_start(wv_sb[:, 512 * db:512 * db + 512],
                                    wvT[128 * db:128 * db + 128, :])
            # scalar queue (idle until exp starts): wq, wk
            for db in range(8):
                nc.scalar.dma_start(wq_sb[:, 512 * db:512 * db + 512],
                                    wqT[128 * db:128 * db + 128, :])
            for db in range(8):
                nc.scalar.dma_start(wk_sb[:, 512 * db:512 * db + 512],
                                    wkT[128 * db:128 * db + 128, :])
            # sync queue: the big x stream in need-order xq, xv, xk
            xq = [xin.tile([128, N], bf16, tag="xin", name=f"xq{db}")
                  for db in range(8)]
            for db in range(8):
                nc.sync.dma_start(xq[db][:], xqT[128 * db:128 * db + 128, :])
            xv = [xin.tile([128, N], bf16, tag="xin", name=f"xv{db}")
                  for db in range(8)]
            for db in range(8):
                nc.sync.dma_start(xv[db][:], xvT[128 * db:128 * db + 128, :])
            xk = [xin.tile([128, N], bf16, tag="xin", name=f"xk{db}")
                  for db in range(8)]
            for db in range(8):
                nc.sync.dma_start(xk[db][:], xkT[128 * db:128 * db + 128, :])

            nc.vector.memset(warm_r[:, :], 1.0)
            nc.vector.memset(vaug_v[:, :, :, 64:128], 1.0)

            # ---- PE warmup: release the HAM clock gate before real work
            wps = pp.tile([128, 512], f32, tag="pp", name="warm")
            for _ in range(WARMUP_MMS):
                nc.tensor.matmul(wps[:], lhsT=warm_r[:, 0:128], rhs=warm_r[:],
                                 start=True, stop=True)

            # ---- unit emitters ----
            def QU(e, tcx):
                ps = pp.tile([128, 512], f32, tag="pp", name=f"q{e}{tcx}")
                for db in range(8):
                    nc.tensor.matmul(
                        ps[:],
                        lhsT=wq_sb[:, 512 * db + 128 * e:512 * db + 128 * e + 128],
                        rhs=xq[db][:, 512 * tcx:512 * tcx + 512],
                        start=(db == 0), stop=(db == 7))
                nc.vector.tensor_scalar(
                    qt_sb[:, 2048 * e + 512 * tcx:2048 * e + 512 * tcx + 512],
                    ps[:], bq_sb[:, e:e + 1], None, add)

            def KU(e, tcx):
                ps = pp.tile([128, 512], f32, tag="pp", name=f"k{e}{tcx}")
                for db in range(8):
                    nc.tensor.matmul(
                        ps[:],
                        lhsT=wk_sb[:, 512 * db + 128 * e:512 * db + 128 * e + 128],
                        rhs=xk[db][:, 512 * tcx:512 * tcx + 512],
                        start=(db == 0), stop=(db == 7))
                nc.vector.tensor_scalar(
                    kt_sb[:, 2048 * e + 512 * tcx:2048 * e + 512 * tcx + 512],
                    ps[:], bk_sb[:, e:e + 1], None, add)

            def VU(tb):
                # V rows for token block tb, all 8 heads
                ps = pp.tile([128, 512], f32, tag="pp", name=f"v{tb}")
                for db in range(8):
                    nc.tensor.matmul(
                        ps[:],
                        lhsT=xv[db][:, 128 * tb:128 * tb + 128],
                        rhs=wv_sb[:, 512 * db:512 * db + 512],
                        start=(db == 0), stop=(db == 7))
                nc.vector.tensor_copy(
                    vaug_v[:, tb, :, 0:64],
                    ps[:, :].rearrange("p (h c) -> p h c", h=8, c=64))

            def OU(t, i, xts):
                # output projection for token block 8*t + i
                ps = pp.tile([128, 512], f32, tag="pp", name=f"o{t}{i}")
                for idx in range(8):
                    nc.tensor.matmul(
                        ps[:],
                        lhsT=xts[idx][:, 128 * i:128 * i + 128],
                        rhs=wo_sb[:, 512 * idx:512 * idx + 512],
                        start=(idx == 0), stop=(idx == 7))
                ysb = ysbp.tile([128, 512], bf16, tag="ysb", name=f"ysb{t}{i}")
                nc.vector.tensor_copy(ysb[:], ps[:])
                tb = 8 * t + i
                nc.sync.dma_start(y[128 * tb:128 * tb + 128, :], ysb[:])

            # ---- prologue: tensor work while the x stream lands ----
            for tcx in range(4):
                QU(0, tcx)
            for tcx in range(4):
                KU(0, tcx)
            for tcx in range(4):
                QU(1, tcx)
            for tcx in range(4):
                QU(2, tcx)
            for tb in range(6):
                VU(tb)
            # deferred wo DMA triggers (gpsimd queue is free now)
            for db in range(8):
                nc.gpsimd.dma_start(wo_sb[:, 512 * db:512 * db + 512],
                                    woT[128 * db:128 * db + 128, :])

            # filler thunks per head-pair block
            fillers = {
                0: [lambda tb=tb: VU(tb) for tb in range(6, NKB)]
                   + [lambda t=t: KU(1, t) for t in range(4)],
                1: [lambda t=t: KU(2, t) for t in range(4)],
                2: [lambda t=t: QU(3, t) for t in range(4)]
                   + [lambda t=t: KU(3, t) for t in range(4)],
                3: [],  # filled dynamically with OU(0, *) after the t0 gather
            }

            xt = {0: [], 1: []}

            def load_xt(t, cis):
                for ci in cis:
                    for r2 in range(2):
                        x = xin.tile([128, N], bf16, tag="xin",
                                     name=f"xt{t}_{ci}_{r2}")
                        if t == 1 and ci == 3:
                            nc.gpsimd.dma_start(
                                x[:, 0:512],
                                cc_out3[0][128 * r2:128 * r2 + 128, :])
                            nc.gpsimd.dma_start(
                                x[:, 512:1024],
                                cc_out3[1][128 * r2:128 * r2 + 128, :])
                        else:
                            nc.gpsimd.dma_start(
                                x[:, 0:1024],
                                cc_out[ci][t][128 * r2:128 * r2 + 128, :])
                        xt[t].append(x)

            # ---- attention streams ----
            def emit_group(e, c, gi):
                hb = 2048 * e
                js = [2 * gi, 2 * gi + 1]
                SG = sgp.tile([128, 4 * QC], f32, tag="SG",
                              name=f"SG{e}_{c}_{gi}")
                for m, j in enumerate(js):
                    for half in (0, 1):
                        po = 64 * half
                        off = 512 * half + QC * m
                        kt_j = kt_sb[po:po + 64,
                                     hb + 128 * j:hb + 128 * j + 128]
                        if j <= 2 * c:
                            nc.tensor.matmul(
                                SG[:, off:off + QC], lhsT=kt_j,
                                rhs=qt_sb[po:po + 64,
                                          hb + QC * c:hb + QC * c + QC],
                                start=True, stop=True,
                                skip_group_check=True)
                        else:  # j == 2c+1: front half is dead
                            nc.tensor.matmul(
                                SG[:, off + 128:off + QC],
                                lhsT=kt_j,
                                rhs=qt_sb[po:po + 64,
                                          hb + QC * c + 128:
                                          hb + QC * c + QC],
                                start=True, stop=True,
                                skip_group_check=True)
                PT = ptp.tile([128, 4 * QC], bf16, tag="PT",
                              name=f"PT{e}_{c}_{gi}")
                nc.scalar.activation(PT[:, :], SG[:, :], Exp, scale=0.125)
                if js[-1] == 2 * c + 1:  # band group: mask diagonal blocks
                    for half in (0, 1):
                        off = 512 * half
                        nc.vector.tensor_tensor(
                            PT[:, off:off + 128],
                            PT[:, off:off + 128], tri_sb[:], mult)
                        nc.vector.tensor_tensor(
                            PT[:, off + QC + 128:off + 2 * QC],
                            PT[:, off + QC + 128:off + 2 * QC],
                            tri_sb[:], mult)
                return (c, js, PT)

            ots_by_chunk = {}

            def emit_pv(e, prev):
                pc, pjs, pPT = prev
                if pjs[0] == 0:
                    OT = otp.tile([128, 2 * QC], f32, tag="OT",
                                  name=f"OT{e}_{pc}")
                    ots_by_chunk[pc] = OT
                OT = ots_by_chunk[pc]
                for m, j in enumerate(pjs):
                    for half in (0, 1):
                        h = 2 * e + half
                        va = vaug_v[:, j, h, 0:128]
                        if j <= 2 * pc:
                            nc.tensor.matmul(
                                OT[:, QC * half:QC * half + QC],
                                lhsT=va,
                                rhs=pPT[:, 512 * half + QC * m:
                                        512 * half + QC * m + QC],
                                # start clears has_written bank-wide: only
                                # the tile's very first matmul may carry it
                                start=(j == 0 and half == 0),
                                stop=(j == 2 * pc + 1),
                                skip_group_check=True)
                        else:  # j == 2pc+1: stream only the live back half
                            nc.tensor.matmul(
                                OT[:, QC * half + 128:QC * half + QC],
                                lhsT=va,
                                rhs=pPT[:, 512 * half + QC * m + 128:
                                        512 * half + QC * m + QC],
                                start=False,
                                stop=(j == 2 * pc + 1),
                                skip_group_check=True)

            def trigger_cc(e, t):
                nc.sync.dma_start(cc_in[e][t][:],
                                  xtown[:, 1024 * t:1024 * t + 1024])
                nc.gpsimd.collective_compute(
                    "AllGather",
                    mybir.AluOpType.bypass,
                    replica_groups=[[0, 1], [2, 3], [4, 5], [6, 7]],
                    ins=[cc_in[e][t].opt()],
                    outs=[cc_out[e][t].opt()],
                )

            def trigger_cc3q(i):
                nc.sync.dma_start(cc_in3[i][:],
                                  xtown[:, 1024 + 512 * i:1536 + 512 * i])
                nc.gpsimd.collective_compute(
                    "AllGather",
                    mybir.AluOpType.bypass,
                    replica_groups=[[0, 1], [2, 3], [4, 5], [6, 7]],
                    ins=[cc_in3[i].opt()],
                    outs=[cc_out3[i].opt()],
                )

            def emit_epilogue(e, pc):
                OT = ots_by_chunk.pop(pc)
                t = pc // 4
                xcol = 1024 * t + QC * (pc % 4)
                # custom-DVE recip mislowers base-partition-64 operands:
                # stage the replicated d rows at base 0 first
                dtm = bcsp.tile([64, 2 * QC], f32, tag="bcs",
                                name=f"dtm{e}_{pc}")
                nc.vector.tensor_copy(dtm[:, :], OT[64:128, :])
                bcs = bcsp.tile([64, 2 * QC], f32, tag="bcs",
                                name=f"bcs{e}_{pc}")
                nc.vector.reciprocal_approx_fast(bcs[:, :], dtm[:, :])
                for half in (0, 1):
                    nc.vector.tensor_tensor(
                        xtown[64 * half:64 * half + 64, xcol:xcol + QC],
                        OT[0:64, QC * half:QC * half + QC],
                        bcs[:, QC * half:QC * half + QC], mult)
                if e == 3 and t == 1:
                    if pc == 5:
                        trigger_cc3q(0)
                    elif pc == 7:
                        trigger_cc3q(1)
                elif pc in (3, 7):
                    trigger_cc(e, t)
                if e == 3 and pc == 3:
                    # final t0 gather fired: its xt pair + the t0 out-proj
                    # queue as filler for chunks 4-7
                    with tc.tile_wait_until(0.215):
                        load_xt(0, [3])
                    fillers[3].extend(
                        [lambda i=i: OU(0, i, xt[0]) for i in range(8)])

            def attention_stream(e):
                groups = [(c, gi) for c in range(8) for gi in range(c + 1)]
                fl = fillers[e]
                fill_idx = 0
                prev = None
                for idx in range(len(groups) + 1):
                    cur = None
                    if idx < len(groups):
                        c, gi = groups[idx]
                        cur = emit_group(e, c, gi)
                    if prev is not None:
                        emit_pv(e, prev)
                        pc, pjs, _ = prev
                        if pjs[-1] == 2 * pc + 1:
                            emit_epilogue(e, pc)
                    # weave fillers evenly across the group stream
                    # (fillers[3] grows mid-stream after the t0 gather)
                    target = (len(fl) * (idx + 1)) // (len(groups) + 1)
                    if e == 3:
                        target = max(0, min(len(fl), (idx + 1 - 14) * 2))
                    while fill_idx < min(target, len(fl)):
                        fl[fill_idx]()
                        fill_idx += 1
                    prev = cur
                while fill_idx < len(fl):
                    fl[fill_idx]()
                    fill_idx += 1

            attention_stream(0)
            attention_stream(1)
            attention_stream(2)
            # gather-dependent loads for the out-projection: all gathers for
            # e0..e2 are done by e3; pin them at e3 logical time so the
            # scheduler cannot hoist their waits ahead of e3's collectives
            with tc.tile_wait_until(0.195):
                load_xt(0, [0, 1, 2])
                load_xt(1, [0, 1, 2])
            attention_stream(3)

            # ---- tail: final quarter gathers -> last xt pair -> t1 out-proj
            with tc.tile_wait_until(0.235):
                load_xt(1, [3])
                for i in range(8):
                    OU(1, i, xt[1])

    nc.compile()
    return nc


def _program():
    global _PROG
    if _PROG is None:
        _PROG = _build_program()
    return _PROG


def _host_inputs(q, k, v, Wq, bq, Wk, bk, Wv, bv, Wo):
    qb = np.asarray(q, np.float32).astype(BF16)
    kb = np.asarray(k, np.float32).astype(BF16)
    vb = np.asarray(v, np.float32).astype(BF16)
    xqT = [np.ascontiguousarray(qb[b].T) for b in range(B)]
    xkT = [np.ascontiguousarray(kb[b].T) for b in range(B)]
    xvT = [np.ascontiguousarray(vb[b].T) for b in range(B)]

    def wslice(W, g):
        return np.ascontiguousarray(
            np.asarray(W, np.float32)[FG * g:FG * (g + 1), :].T).astype(BF16)

    wqg = [wslice(Wq, g) for g in range(2)]
    wkg = [wslice(Wk, g) for g in range(2)]
    wvg = [wslice(Wv, g) for g in range(2)]

    def woslice(g):
        wt = np.ascontiguousarray(
            np.asarray(Wo, np.float32)[FG * g:FG * (g + 1), :].T).astype(BF16)
        # permute 128-row input-feature blocks to the chunked-AG order
        return np.ascontiguousarray(
            wt.reshape(8, 128, FG)[PERM].reshape(D, FG))

    wog = [woslice(g) for g in range(2)]

    def bslice(bvec, g):
        return np.ascontiguousarray(
            np.asarray(bvec, np.float32)[FG * g:FG * (g + 1)]
            .reshape(4, 128).T)

    bqg = [bslice(bq, g) for g in range(2)]
    bkg = [bslice(bk, g) for g in range(2)]

    kk, qq = np.meshgrid(np.arange(128), np.arange(128), indexing="ij")
    tri = np.where(kk <= qq, 1.0, 0.0).astype(BF16)

    in_maps = []
    for core in range(N_CORES):
        b, g = core // 2, core % 2
        in_maps.append({
            "xqT": xqT[b], "xkT": xkT[b], "xvT": xvT[b],
            "wqT": wqg[g], "wkT": wkg[g], "wvT": wvg[g], "woT": wog[g],
            "bq2": bqg[g], "bk2": bkg[g], "tri01": tri,
        })
    return in_maps


def run_sharded(in_maps, trace=False, trace_kwargs=None):
    from concourse.bass_utils import run_bass_kernel_spmd
    nc = _program()
    return run_bass_kernel_spmd(nc, in_maps, core_ids=list(range(N_CORES)),
                                trace=trace, trace_kwargs=trace_kwargs or {})


def kernel(q, k, v, Wq, bq, Wk, bk, Wv, bv, Wo):
    in_maps = _host_inputs(q, k, v, Wq, bq, Wk, bk, Wv, bv, Wo)
    res = run_sharded(in_maps)
    out = np.empty((B, N, D), np.float32)
    for b in range(B):
        out[b, :, 0:FG] = res.results[2 * b]["y"]
        out[b, :, FG:D] = res.results[2 * b + 1]["y"]
    return out
